# revision 28
# baseline (speedup 1.0000x reference)
"""Trainium2 Bass kernel for nn_MultiHeadAttention (B=8192, D=1024, 16 heads
used only via the softmax scale 1/8).

Strategy (8 NeuronCores, sharded projections + pipelined AllGather):
  - Rows (batch axis) are sharded: core c owns rows [c*1024, (c+1)*1024).
  - Each core computes Q^T, K^T and V for ONLY its local 1024 rows
    (6.4 GFLOP/core instead of the 36 GFLOP/core a replicated K/V
    projection would cost). The K^T and V shards are AllGathered in TWO
    halves (key rows 0:512 and 512:1024 of each rank, bf16), each
    triggered as soon as it is computed so the collectives (TOPSP/SDMA
    silicon) overlap Q^T and the start of attention.
  - Attention runs in a transposed-energy ("E^T") layout so no probability
    transpose is ever needed:
        E^T[j, i] = sum_o K^T[o, j] * Q^T[o, i]
        P^T = exp(E^T * 0.125)            (no max subtraction; safe in f32)
        out_unnorm[i, o] = sum_j P^T[j, i] * V[j, o]
        s[i] = sum_j P^T[j, i]            (matmul against a ones vector)
        out = out_unnorm / s + bv         (bv folded in post-normalization)
  - K^T travels bf16 over the wire but is upcast to f32r on load so the
    E^T matmul runs f32r x f32r; P/V stay bf16.
  - Phase 2 runs in ring order: the core's OWN two j-blocks first, read
    from its local shard (no collective dependency — fills the collective
    startup latency), then ranks (pid+s) mod 8 via register-offset DMAs
    into the gathered buffers, off=0 half before off=512 half.
"""

import sys

sys.path.insert(0, "/opt/trn_rl_repo")

import numpy as np

import concourse.bass as bass  # noqa: F401
import concourse.tile as tile
from concourse import bacc, mybir
from concourse.bass import ds
from concourse.bass_utils import run_bass_kernel_spmd
from concourse.masks import make_identity

B = 8192
D = 1024
P = 128
NCORES = 8
R = B // NCORES  # 1024 rows per core
JBLK = 512  # j-block (keys/values) streamed per iteration
NJB = B // JBLK  # 16
DO = D // P  # 8 feature chunks of 128
IC = R // P  # 8 row chunks of 128 per core
F32 = mybir.dt.float32
F32R = mybir.dt.float32r
BF16 = mybir.dt.bfloat16
AF = mybir.ActivationFunctionType
ALU = mybir.AluOpType
SCALE = 0.125  # 1/sqrt(head_dim=64)
RG = [list(range(NCORES))]
HALF = DO * P * JBLK  # flat bf16 elements of one K or V half (512K)


def _transpose_weight(nc, tp_psum, row_pool, identity_r, w_dram, wt_sb):
    """PE-transpose a [D, D] weight into the [128(d_in), DO, D(out)] SBUF
    layout (wt_sb[:, dd, o] = W[o, dd*128 + p])."""
    for oo in range(DO):
        wrow = row_pool.tile([P, D], F32, tag="row", name="wrow")
        nc.sync.dma_start(wrow, w_dram[oo * P : (oo + 1) * P, :])
        wrow_r = row_pool.tile([P, D], F32R, tag="rowr", name="wrow_r")
        if oo % 2 == 0:
            nc.scalar.activation(wrow_r, wrow, AF.Identity)
        else:
            nc.vector.tensor_copy(out=wrow_r, in_=wrow)
        for d4 in range(DO // 4):
            tp4 = tp_psum.tile([P, 4, P], F32, tag="tp", name="tp4")
            for q in range(4):
                dd = d4 * 4 + q
                nc.tensor.matmul(
                    tp4[:, q, :],
                    wrow_r[:, dd * P : (dd + 1) * P],
                    identity_r,
                    start=True,
                    stop=True,
                )
            nc.vector.tensor_copy(
                out=wt_sb[:, d4 * 4 : (d4 + 1) * 4, oo * P : (oo + 1) * P],
                in_=tp4,
            )


def build_program():
    nc = bacc.Bacc(
        "TRN2", target_bir_lowering=False, debug=False, num_devices=NCORES
    )
    x_loc = nc.dram_tensor("x_loc", [R, D], F32, kind="ExternalInput").ap()
    w_q = nc.dram_tensor("Wq", [D, D], F32, kind="ExternalInput").ap()
    w_k = nc.dram_tensor("Wk", [D, D], F32, kind="ExternalInput").ap()
    w_v = nc.dram_tensor("Wv", [D, D], F32, kind="ExternalInput").ap()
    b_q = nc.dram_tensor("bq", [D], F32, kind="ExternalInput").ap()
    b_k = nc.dram_tensor("bk", [D], F32, kind="ExternalInput").ap()
    b_v = nc.dram_tensor("bv", [D], F32, kind="ExternalInput").ap()
    out_loc = nc.dram_tensor("out_loc", [R, D], F32, kind="ExternalOutput").ap()

    with tile.TileContext(nc) as tc:
        _body(nc, tc, x_loc, w_q, w_k, w_v, b_q, b_k, b_v, out_loc)
    nc.compile()
    return nc


def _body(nc, tc, x_loc, w_q, w_k, w_v, b_q, b_k, b_v, out_loc):
    from contextlib import ExitStack

    outer = ExitStack()
    outer.__enter__()
    # ---- persistent pools (whole kernel) ----
    const_pool = outer.enter_context(tc.tile_pool(name="const", bufs=1))
    identity = const_pool.tile([P, P], F32)
    make_identity(nc, identity)
    id_r = const_pool.tile([P, P], F32R)
    nc.vector.tensor_copy(out=id_r, in_=identity)
    ones_f32 = const_pool.tile([P, 2], F32)
    nc.vector.memset(ones_f32, 1.0)
    ones = const_pool.tile([P, 2], BF16)
    nc.vector.tensor_copy(out=ones, in_=ones_f32)
    bq_sb = const_pool.tile([P, DO], F32)
    nc.sync.dma_start(bq_sb, b_q.rearrange("(oo p) -> p oo", p=P))
    bk_sb = const_pool.tile([P, DO], F32)
    nc.sync.dma_start(bk_sb, b_k.rearrange("(oo p) -> p oo", p=P))
    ones_row = const_pool.tile([1, P], F32)
    nc.vector.memset(ones_row, 1.0)
    # broadcast bv across all 128 partitions with a K=1 matmul:
    # load bv into partition 0 of bv_bc, then out[p, o] = 1 * bv[o]
    bv_bc = const_pool.tile([P, D], F32)
    nc.sync.dma_start(bv_bc[0:1, :], b_v[None, :])
    with tc.tile_pool(name="bv_psum", bufs=2, space="PSUM") as bvp:
        for oh in range(2):
            pt = bvp.tile([P, 512], F32, tag="bvp")
            nc.tensor.matmul(
                pt,
                ones_row,
                bv_bc[0:1, oh * 512 : (oh + 1) * 512],
                start=True,
                stop=True,
            )
            nc.vector.tensor_copy(out=bv_bc[:, oh * 512 : (oh + 1) * 512], in_=pt)

    qt_pool = outer.enter_context(tc.tile_pool(name="qt", bufs=1))
    qt = qt_pool.tile([P, DO, R], F32R)  # Q^T: [o_in, o_out, i]  (4 MB)
    # the local K^T / V shard stays resident in SBUF so the own-rank
    # j-blocks of phase 2 need no DMA (and no collective) at all
    kvl_pool = outer.enter_context(tc.tile_pool(name="kvl", bufs=1))
    klocal = kvl_pool.tile([P, DO, R], BF16)  # K^T local (2 MB)
    vlocal = kvl_pool.tile([P, IC, D], BF16)  # V local, [j%128, j//128, o] (2 MB)

    sums_pool = outer.enter_context(tc.tile_pool(name="sums", bufs=1))
    sums_acc = sums_pool.tile([P, 2 * IC], F32)  # per-row exp-sums (even cols)
    rsum = sums_pool.tile([P, 2 * IC], F32)

    # DRAM scratch: per-half local K^T+V shard (collective inputs) and the
    # AllGathered halves (flat, indexed dynamically by ring position).
    # Each half is [2, HALF] bf16: slot 0 = K^T [DO, P, JBLK] flattened,
    # slot 1 = V rows [JBLK, D] flattened.
    dram = outer.enter_context(tc.tile_pool(name="dram", bufs=1, space="DRAM"))
    kv_loc_h = [dram.tile([2, HALF], BF16, name=f"kv_loc_{h}") for h in range(2)]
    kv_full_h = [
        dram.tile(
            [NCORES * 2 * HALF], BF16, addr_space="Shared", name=f"kv_full_{h}"
        )
        for h in range(2)
    ]

    def k_dst(h, oo):  # [P, JBLK] write view of K^T half h, feature chunk oo
        return kv_loc_h[h][0].rearrange("(a p r) -> a p r", a=DO, p=P)[oo]

    def v_dst(h, jv):  # [P, D] write view of V half h, row chunk jv
        return kv_loc_h[h][1].rearrange("(j o) -> j o", o=D)[
            jv * P : (jv + 1) * P, :
        ]

    # =========================================================
    # Phase 1: weight transposes, local x^T, local K^T/V/Q^T,
    #          two pipelined AllGathers of the K^T+V halves
    # =========================================================
    with ExitStack() as p1:
        row_pool = p1.enter_context(tc.tile_pool(name="rows", bufs=3))
        xt_pool = p1.enter_context(tc.tile_pool(name="xt", bufs=1))
        tp_psum = p1.enter_context(tc.tile_pool(name="tp_ps", bufs=2, space="PSUM"))
        mm_psum = p1.enter_context(tc.tile_pool(name="mm_ps", bufs=6, space="PSUM"))

        xt = xt_pool.tile([P, DO, R], F32R)  # x^T local: [d_in, d_out, i] 4MB

        with ExitStack() as pkv:
            wt_pool = pkv.enter_context(tc.tile_pool(name="wt", bufs=1))
            wkt = wt_pool.tile([P, DO, D], F32R)  # W^T: [d_in, d_out, o] 4MB
            wvt = wt_pool.tile([P, DO, D], F32R)

            # -- transpose Wk; transpose the local x rows; transpose Wv --
            _transpose_weight(nc, tp_psum, row_pool, id_r, w_k, wkt)
            for jj in range(IC):
                xrow = row_pool.tile([P, D], F32, tag="row", name="xrow")
                nc.sync.dma_start(xrow, x_loc[jj * P : (jj + 1) * P, :])
                xrow_r = row_pool.tile([P, D], F32R, tag="rowr", name="xrow_r")
                if jj % 2 == 0:
                    nc.scalar.activation(xrow_r, xrow, AF.Identity)
                else:
                    nc.vector.tensor_copy(out=xrow_r, in_=xrow)
                for d4 in range(DO // 4):
                    tp4 = tp_psum.tile([P, 4, P], F32, tag="tp", name="tp4")
                    for q in range(4):
                        dd = d4 * 4 + q
                        nc.tensor.matmul(
                            tp4[:, q, :],
                            xrow_r[:, dd * P : (dd + 1) * P],
                            id_r,
                            start=True,
                            stop=True,
                        )
                    nc.vector.tensor_copy(
                        out=xt[:, d4 * 4 : (d4 + 1) * 4, jj * P : (jj + 1) * P],
                        in_=tp4,
                    )
            _transpose_weight(nc, tp_psum, row_pool, id_r, w_v, wvt)

            # -- per half: local K^T columns (into SBUF-resident klocal),
            #    local V rows (into vlocal), DMA out, then AllGather --
            for h in range(2):
                for oo in range(DO):
                    pk = mm_psum.tile([P, JBLK], F32, tag="mm", name="pk")
                    for dd in range(DO):
                        nc.tensor.matmul(
                            pk,
                            (wkt[:, dd, oo * P : (oo + 1) * P]),
                            (xt[:, dd, h * JBLK : (h + 1) * JBLK]),
                            start=(dd == 0),
                            stop=(dd == DO - 1),
                        )
                    nc.scalar.activation(
                        klocal[:, oo, h * JBLK : (h + 1) * JBLK],
                        pk,
                        AF.Identity,
                        bias=bk_sb[:, oo : oo + 1],
                    )
                    nc.sync.dma_start(
                        k_dst(h, oo), klocal[:, oo, h * JBLK : (h + 1) * JBLK]
                    )
                for jv in range(JBLK // P):
                    jj = h * (JBLK // P) + jv
                    pv_h = [
                        mm_psum.tile([P, JBLK], F32, tag="mm", name="pv")
                        for _ in range(2)
                    ]
                    for dd in range(DO):
                        for oh in range(2):
                            nc.tensor.matmul(
                                pv_h[oh],
                                (xt[:, dd, jj * P : (jj + 1) * P]),
                                (wvt[:, dd, oh * 512 : (oh + 1) * 512]),
                                start=(dd == 0),
                                stop=(dd == DO - 1),
                            )
                    for oh in range(2):
                        nc.vector.tensor_copy(
                            out=vlocal[:, jj, oh * 512 : (oh + 1) * 512],
                            in_=pv_h[oh],
                        )
                    nc.sync.dma_start(v_dst(h, jv), vlocal[:, jj, :])
                nc.gpsimd.collective_compute(
                    "AllGather",
                    ALU.bypass,
                    replica_groups=RG,
                    ins=[kv_loc_h[h].opt()],
                    outs=[kv_full_h[h].opt()],
                )

        # -- local Q^T (overlaps the collectives) --
        wq_pool = p1.enter_context(tc.tile_pool(name="wq", bufs=1))
        wqt = wq_pool.tile([P, DO, D], F32R)
        _transpose_weight(nc, tp_psum, row_pool, id_r, w_q, wqt)
        for oo in range(DO):
            pq_h = [
                mm_psum.tile([P, JBLK], F32, tag="mm", name="pq") for _ in range(2)
            ]
            for dd in range(DO):
                for ih in range(2):
                    nc.tensor.matmul(
                        pq_h[ih],
                        (wqt[:, dd, oo * P : (oo + 1) * P]),
                        (xt[:, dd, ih * JBLK : (ih + 1) * JBLK]),
                        start=(dd == 0),
                        stop=(dd == DO - 1),
                    )
            for ih in range(2):
                nc.scalar.activation(
                    qt[:, oo, ih * JBLK : (ih + 1) * JBLK],
                    pq_h[ih],
                    AF.Identity,
                    bias=bq_sb[:, oo : oo + 1],
                )

    # =========================================================
    # Phase 2: streamed attention in E^T layout, ring order.
    # Block order: own shard (both halves, from kv_loc, no AG dep),
    # then ranks (pid+s) mod 8 for s=1..7, off=0 half then off=512.
    # =========================================================
    with ExitStack() as p2:
        oa_pool = p2.enter_context(tc.tile_pool(name="oacc", bufs=1))
        outacc = oa_pool.tile([P, IC, D], F32)  # 4 MB

        ktb_bf_pool = p2.enter_context(tc.tile_pool(name="ktbb", bufs=2))
        kt_pool = p2.enter_context(tc.tile_pool(name="ktb", bufs=2))
        v_pool = p2.enter_context(tc.tile_pool(name="vtb", bufs=3))
        pt_pool = p2.enter_context(tc.tile_pool(name="ptb", bufs=3))
        e_psum = p2.enter_context(tc.tile_pool(name="e_ps", bufs=4, space="PSUM"))
        o_psum = p2.enter_context(tc.tile_pool(name="o_ps", bufs=3, space="PSUM"))
        s_psum = p2.enter_context(tc.tile_pool(name="s_ps", bufs=1, space="PSUM"))

        fin_pool = p2.enter_context(tc.tile_pool(name="fin", bufs=2))
        pid = nc.sync.partition_id()

        blocks = [("loc", 0, 0), ("loc", 1, 0)]
        blocks += [("rem", h, s) for h in (0, 1) for s in range(1, NCORES)]

        for t, (kind, h, s) in enumerate(blocks):
            ktb = kt_pool.tile([P, DO, JBLK], F32R, tag="ktb")
            if kind == "loc":
                # own shard: upcast straight out of SBUF, no DMA at all
                for oo in range(DO):
                    nc.vector.tensor_copy(
                        out=ktb[:, oo, :],
                        in_=klocal[:, oo, h * JBLK : (h + 1) * JBLK],
                    )
                vtb = vlocal[:, h * (JBLK // P) : (h + 1) * (JBLK // P), :]
            else:
                rr = (pid + s) & (NCORES - 1)
                k_src = kv_full_h[h][ds(rr * (2 * HALF), HALF)].rearrange(
                    "(a p r) -> p a r", a=DO, p=P
                )
                v_src = kv_full_h[h][ds(rr * (2 * HALF) + HALF, HALF)].rearrange(
                    "(jj p o) -> p jj o", p=P, o=D
                )
                ktb_bf = ktb_bf_pool.tile([P, DO, JBLK], BF16, tag="ktbb")
                for oo in range(DO):
                    nc.sync.dma_start(ktb_bf[:, oo, :], k_src[:, oo, :])
                    nc.vector.tensor_copy(out=ktb[:, oo, :], in_=ktb_bf[:, oo, :])
                vtb = v_pool.tile([P, JBLK // P, D], BF16, tag="vtb")
                nc.sync.dma_start(vtb, v_src)
            # unnormalized probabilities P^T for this j-block: [j, i]
            ptb = pt_pool.tile([P, JBLK // P, R], BF16, tag="ptb")
            for jj in range(JBLK // P):
                pe_h = [
                    e_psum.tile([P, JBLK], F32, tag="pe", name="pe")
                    for _ in range(R // JBLK)
                ]
                for oo in range(DO):
                    for ih in range(R // JBLK):
                        nc.tensor.matmul(
                            pe_h[ih],
                            (ktb[:, oo, jj * P : (jj + 1) * P]),
                            (qt[:, oo, ih * JBLK : (ih + 1) * JBLK]),
                            start=(oo == 0),
                            stop=(oo == DO - 1),
                        )
                for ih in range(R // JBLK):
                    nc.scalar.activation(
                        ptb[:, jj, ih * JBLK : (ih + 1) * JBLK],
                        pe_h[ih],
                        AF.Exp,
                        scale=SCALE,
                    )
            # row sums of P^T (reduce over j): matmul against ones
            # out_unnorm += P^T.T @ V, with the exp-sums matmul sharing each
            # stationary ptb tile (3 streams per weight load)
            last = t == len(blocks) - 1
            ps = s_psum.tile([P, 2 * IC], F32, tag="ps")
            for ic in range(IC):
                po_h = [o_psum.tile([P, 512], F32, tag="po", name="po") for _ in range(2)]
                for jj in range(JBLK // P):
                    for oh in range(2):
                        nc.tensor.matmul(
                            po_h[oh],
                            (ptb[:, jj, ic * P : (ic + 1) * P]),
                            (vtb[:, jj, oh * 512 : (oh + 1) * 512]),
                            start=(jj == 0),
                            stop=(jj == JBLK // P - 1),
                        )
                    # sums group: whole-block on most blocks; per-ic on the
                    # last block so the epilogue pipelines with the PE
                    nc.tensor.matmul(
                        ps[:, 2 * ic : 2 * ic + 2],
                        (ptb[:, jj, ic * P : (ic + 1) * P]),
                        (ones),
                        start=(jj == 0) if last else (ic == 0 and jj == 0),
                        stop=(jj == JBLK // P - 1)
                        if last
                        else (ic == IC - 1 and jj == JBLK // P - 1),
                    )
                for oh in range(2):
                    dst = outacc[:, ic, oh * 512 : (oh + 1) * 512]
                    if t == 0:
                        nc.vector.tensor_copy(out=dst, in_=po_h[oh])
                    else:
                        nc.vector.tensor_tensor(dst, po_h[oh], dst, ALU.add)
                if last:
                    # finalize rows [ic*128, (ic+1)*128) while the PE moves on
                    nc.vector.tensor_tensor(
                        sums_acc[:, 2 * ic : 2 * ic + 2],
                        ps[:, 2 * ic : 2 * ic + 2],
                        sums_acc[:, 2 * ic : 2 * ic + 2],
                        ALU.add,
                    )
                    nc.vector.reciprocal(
                        rsum[:, 2 * ic : 2 * ic + 2],
                        sums_acc[:, 2 * ic : 2 * ic + 2],
                    )
                    ofin = fin_pool.tile([P, D], F32, tag="ofin", name="ofin")
                    nc.vector.tensor_scalar_mul(
                        ofin, outacc[:, ic, :], rsum[:, 2 * ic : 2 * ic + 1]
                    )
                    nc.vector.tensor_tensor(ofin, ofin, bv_bc, ALU.add)
                    nc.sync.dma_start(out_loc[ic * P : (ic + 1) * P, :], ofin)
            if not last:
                if t == 0:
                    nc.vector.tensor_copy(out=sums_acc, in_=ps)
                else:
                    nc.vector.tensor_tensor(sums_acc, ps, sums_acc, ALU.add)

    outer.close()


_NC_CACHE = None


def _get_program():
    global _NC_CACHE
    if _NC_CACHE is None:
        _NC_CACHE = build_program()
    return _NC_CACHE


def _run(inputs, trace=False):
    nc = _get_program()
    x = np.ascontiguousarray(np.asarray(inputs["x"], dtype=np.float32))
    common = {
        k: np.ascontiguousarray(np.asarray(inputs[k], dtype=np.float32))
        for k in ("Wq", "Wk", "Wv", "bq", "bk", "bv")
    }
    in_maps = [
        {"x_loc": np.ascontiguousarray(x[c * R : (c + 1) * R]), **common}
        for c in range(NCORES)
    ]
    try:
        res = run_bass_kernel_spmd(
            nc, in_maps, core_ids=list(range(NCORES)), trace=trace
        )
    except Exception:
        # transient NRT_EXEC_UNIT_UNRECOVERABLE wedges recover on retry
        import time as _time

        _time.sleep(2)
        res = run_bass_kernel_spmd(
            nc, in_maps, core_ids=list(range(NCORES)), trace=trace
        )
    out = np.concatenate([res.results[c]["out_loc"] for c in range(NCORES)], axis=0)
    return out.reshape(B, D, 1).astype(np.float32), res


def kernel(**inputs):
    out, _ = _run(inputs, trace=False)
    return out


# revision 29
# speedup vs baseline: 1.0156x; 1.0156x over previous
"""Trainium2 Bass kernel for nn_MultiHeadAttention (B=8192, D=1024, 16 heads
used only via the softmax scale 1/8).

Strategy (8 NeuronCores, sharded projections + pipelined AllGather):
  - Rows (batch axis) are sharded: core c owns rows [c*1024, (c+1)*1024).
  - Each core computes Q^T, K^T and V for ONLY its local 1024 rows
    (6.4 GFLOP/core instead of the 36 GFLOP/core a replicated K/V
    projection would cost). The K^T and V shards are AllGathered in TWO
    halves (key rows 0:512 and 512:1024 of each rank, bf16), each
    triggered as soon as it is computed so the collectives (TOPSP/SDMA
    silicon) overlap Q^T and the start of attention.
  - Attention runs in a transposed-energy ("E^T") layout so no probability
    transpose is ever needed:
        E^T[j, i] = sum_o K^T[o, j] * Q^T[o, i]
        P^T = exp(E^T * 0.125)            (no max subtraction; safe in f32)
        out_unnorm[i, o] = sum_j P^T[j, i] * V[j, o]
        s[i] = sum_j P^T[j, i]            (matmul against a ones vector)
        out = out_unnorm / s + bv         (bv folded in post-normalization)
  - K^T travels bf16 over the wire but is upcast to f32r on load so the
    E^T matmul runs f32r x f32r; P/V stay bf16.
  - Phase 2 runs in ring order: the core's OWN two j-blocks first, read
    from its local shard (no collective dependency — fills the collective
    startup latency), then ranks (pid+s) mod 8 via register-offset DMAs
    into the gathered buffers, off=0 half before off=512 half.
"""

import sys

sys.path.insert(0, "/opt/trn_rl_repo")

import numpy as np

import concourse.bass as bass  # noqa: F401
import concourse.tile as tile
from concourse import bacc, mybir
from concourse.bass import ds
from concourse.bass_utils import run_bass_kernel_spmd
from concourse.masks import make_identity

B = 8192
D = 1024
P = 128
NCORES = 8
R = B // NCORES  # 1024 rows per core
JBLK = 512  # j-block (keys/values) streamed per iteration
NJB = B // JBLK  # 16
DO = D // P  # 8 feature chunks of 128
IC = R // P  # 8 row chunks of 128 per core
F32 = mybir.dt.float32
F32R = mybir.dt.float32r
BF16 = mybir.dt.bfloat16
AF = mybir.ActivationFunctionType
ALU = mybir.AluOpType
SCALE = 0.125  # 1/sqrt(head_dim=64)
RG = [list(range(NCORES))]
HALF = DO * P * JBLK  # flat bf16 elements of one K or V half (512K)


def _transpose_weight(nc, tp_psum, row_pool, identity_r, w_dram, wt_sb):
    """PE-transpose a [D, D] weight into the [128(d_in), DO, D(out)] SBUF
    layout (wt_sb[:, dd, o] = W[o, dd*128 + p])."""
    for oo in range(DO):
        wrow = row_pool.tile([P, D], F32, tag="row", name="wrow")
        nc.sync.dma_start(wrow, w_dram[oo * P : (oo + 1) * P, :])
        wrow_r = row_pool.tile([P, D], F32R, tag="rowr", name="wrow_r")
        if oo % 2 == 0:
            nc.scalar.activation(wrow_r, wrow, AF.Identity)
        else:
            nc.vector.tensor_copy(out=wrow_r, in_=wrow)
        for d4 in range(DO // 4):
            tp4 = tp_psum.tile([P, 4, P], F32, tag="tp", name="tp4")
            for q in range(4):
                dd = d4 * 4 + q
                nc.tensor.matmul(
                    tp4[:, q, :],
                    wrow_r[:, dd * P : (dd + 1) * P],
                    identity_r,
                    start=True,
                    stop=True,
                )
            nc.vector.tensor_copy(
                out=wt_sb[:, d4 * 4 : (d4 + 1) * 4, oo * P : (oo + 1) * P],
                in_=tp4,
            )


def build_program():
    nc = bacc.Bacc(
        "TRN2", target_bir_lowering=False, debug=False, num_devices=NCORES
    )
    x_loc = nc.dram_tensor("x_loc", [R, D], F32, kind="ExternalInput").ap()
    w_q = nc.dram_tensor("Wq", [D, D], F32, kind="ExternalInput").ap()
    w_k = nc.dram_tensor("Wk", [D, D], F32, kind="ExternalInput").ap()
    w_v = nc.dram_tensor("Wv", [D, D], F32, kind="ExternalInput").ap()
    b_q = nc.dram_tensor("bq", [D], F32, kind="ExternalInput").ap()
    b_k = nc.dram_tensor("bk", [D], F32, kind="ExternalInput").ap()
    b_v = nc.dram_tensor("bv", [D], F32, kind="ExternalInput").ap()
    out_loc = nc.dram_tensor("out_loc", [R, D], F32, kind="ExternalOutput").ap()

    with tile.TileContext(nc) as tc:
        _body(nc, tc, x_loc, w_q, w_k, w_v, b_q, b_k, b_v, out_loc)
    nc.compile()
    return nc


def _body(nc, tc, x_loc, w_q, w_k, w_v, b_q, b_k, b_v, out_loc):
    from contextlib import ExitStack

    outer = ExitStack()
    outer.__enter__()
    # ---- persistent pools (whole kernel) ----
    const_pool = outer.enter_context(tc.tile_pool(name="const", bufs=1))
    identity = const_pool.tile([P, P], F32)
    make_identity(nc, identity)
    id_r = const_pool.tile([P, P], F32R)
    nc.vector.tensor_copy(out=id_r, in_=identity)
    ones_f32 = const_pool.tile([P, 2], F32)
    nc.vector.memset(ones_f32, 1.0)
    ones = const_pool.tile([P, 2], BF16)
    nc.vector.tensor_copy(out=ones, in_=ones_f32)
    bq_sb = const_pool.tile([P, DO], F32)
    nc.sync.dma_start(bq_sb, b_q.rearrange("(oo p) -> p oo", p=P))
    bk_sb = const_pool.tile([P, DO], F32)
    nc.sync.dma_start(bk_sb, b_k.rearrange("(oo p) -> p oo", p=P))
    ones_row = const_pool.tile([1, P], F32)
    nc.vector.memset(ones_row, 1.0)
    # broadcast bv across all 128 partitions with a K=1 matmul:
    # load bv into partition 0 of bv_bc, then out[p, o] = 1 * bv[o]
    bv_bc = const_pool.tile([P, D], F32)
    nc.sync.dma_start(bv_bc[0:1, :], b_v[None, :])
    with tc.tile_pool(name="bv_psum", bufs=2, space="PSUM") as bvp:
        for oh in range(2):
            pt = bvp.tile([P, 512], F32, tag="bvp")
            nc.tensor.matmul(
                pt,
                ones_row,
                bv_bc[0:1, oh * 512 : (oh + 1) * 512],
                start=True,
                stop=True,
            )
            nc.vector.tensor_copy(out=bv_bc[:, oh * 512 : (oh + 1) * 512], in_=pt)

    qt_pool = outer.enter_context(tc.tile_pool(name="qt", bufs=1))
    qt = qt_pool.tile([P, DO, R], F32R)  # Q^T: [o_in, o_out, i]  (4 MB)
    # the local K^T / V shard stays resident in SBUF so the own-rank
    # j-blocks of phase 2 need no DMA (and no collective) at all
    kvl_pool = outer.enter_context(tc.tile_pool(name="kvl", bufs=1))
    klocal = kvl_pool.tile([P, DO, R], BF16)  # K^T local (2 MB)
    vlocal = kvl_pool.tile([P, IC, D], BF16)  # V local, [j%128, j//128, o] (2 MB)

    sums_pool = outer.enter_context(tc.tile_pool(name="sums", bufs=1))
    sums_acc = sums_pool.tile([P, 2 * IC], F32)  # per-row exp-sums (even cols)
    rsum = sums_pool.tile([P, 2 * IC], F32)

    # DRAM scratch: per-half local K^T+V shard (collective inputs) and the
    # AllGathered halves (flat, indexed dynamically by ring position).
    # Each half is [2, HALF] bf16: slot 0 = K^T [DO, P, JBLK] flattened,
    # slot 1 = V rows [JBLK, D] flattened.
    dram = outer.enter_context(tc.tile_pool(name="dram", bufs=1, space="DRAM"))
    kv_loc_h = [dram.tile([2, HALF], BF16, name=f"kv_loc_{h}") for h in range(2)]
    kv_full_h = [
        dram.tile(
            [NCORES * 2 * HALF], BF16, addr_space="Shared", name=f"kv_full_{h}"
        )
        for h in range(2)
    ]

    def k_dst(h, oo):  # [P, JBLK] write view of K^T half h, feature chunk oo
        return kv_loc_h[h][0].rearrange("(a p r) -> a p r", a=DO, p=P)[oo]

    def v_dst(h, jv):  # [P, D] write view of V half h, row chunk jv
        return kv_loc_h[h][1].rearrange("(j o) -> j o", o=D)[
            jv * P : (jv + 1) * P, :
        ]

    # =========================================================
    # Phase 1: weight transposes, local x^T, local K^T/V/Q^T,
    #          two pipelined AllGathers of the K^T+V halves
    # =========================================================
    with ExitStack() as p1:
        row_pool = p1.enter_context(tc.tile_pool(name="rows", bufs=3))
        xt_pool = p1.enter_context(tc.tile_pool(name="xt", bufs=1))
        tp_psum = p1.enter_context(tc.tile_pool(name="tp_ps", bufs=2, space="PSUM"))
        mm_psum = p1.enter_context(tc.tile_pool(name="mm_ps", bufs=6, space="PSUM"))

        xt = xt_pool.tile([P, DO, R], F32R)  # x^T local: [d_in, d_out, i] 4MB

        with ExitStack() as pkv:
            wt_pool = pkv.enter_context(tc.tile_pool(name="wt", bufs=1))
            wkt = wt_pool.tile([P, DO, D], F32R)  # W^T: [d_in, d_out, o] 4MB
            wvt = wt_pool.tile([P, DO, D], F32R)

            # -- transpose Wk; transpose the local x rows; transpose Wv --
            _transpose_weight(nc, tp_psum, row_pool, id_r, w_k, wkt)
            for jj in range(IC):
                xrow = row_pool.tile([P, D], F32, tag="row", name="xrow")
                nc.sync.dma_start(xrow, x_loc[jj * P : (jj + 1) * P, :])
                xrow_r = row_pool.tile([P, D], F32R, tag="rowr", name="xrow_r")
                if jj % 2 == 0:
                    nc.scalar.activation(xrow_r, xrow, AF.Identity)
                else:
                    nc.vector.tensor_copy(out=xrow_r, in_=xrow)
                for d4 in range(DO // 4):
                    tp4 = tp_psum.tile([P, 4, P], F32, tag="tp", name="tp4")
                    for q in range(4):
                        dd = d4 * 4 + q
                        nc.tensor.matmul(
                            tp4[:, q, :],
                            xrow_r[:, dd * P : (dd + 1) * P],
                            id_r,
                            start=True,
                            stop=True,
                        )
                    nc.vector.tensor_copy(
                        out=xt[:, d4 * 4 : (d4 + 1) * 4, jj * P : (jj + 1) * P],
                        in_=tp4,
                    )
            _transpose_weight(nc, tp_psum, row_pool, id_r, w_v, wvt)

            # -- per half: local K^T columns (into SBUF-resident klocal),
            #    local V rows (into vlocal), DMA out, then AllGather --
            for h in range(2):
                for oo in range(DO):
                    pk = mm_psum.tile([P, JBLK], F32, tag="mm", name="pk")
                    for dd in range(DO):
                        nc.tensor.matmul(
                            pk,
                            (wkt[:, dd, oo * P : (oo + 1) * P]),
                            (xt[:, dd, h * JBLK : (h + 1) * JBLK]),
                            start=(dd == 0),
                            stop=(dd == DO - 1),
                        )
                    nc.scalar.activation(
                        klocal[:, oo, h * JBLK : (h + 1) * JBLK],
                        pk,
                        AF.Identity,
                        bias=bk_sb[:, oo : oo + 1],
                    )
                    nc.sync.dma_start(
                        k_dst(h, oo), klocal[:, oo, h * JBLK : (h + 1) * JBLK]
                    )
                for jv in range(JBLK // P):
                    jj = h * (JBLK // P) + jv
                    pv_h = [
                        mm_psum.tile([P, JBLK], F32, tag="mm", name="pv")
                        for _ in range(2)
                    ]
                    for dd in range(DO):
                        for oh in range(2):
                            nc.tensor.matmul(
                                pv_h[oh],
                                (xt[:, dd, jj * P : (jj + 1) * P]),
                                (wvt[:, dd, oh * 512 : (oh + 1) * 512]),
                                start=(dd == 0),
                                stop=(dd == DO - 1),
                            )
                    for oh in range(2):
                        nc.vector.tensor_copy(
                            out=vlocal[:, jj, oh * 512 : (oh + 1) * 512],
                            in_=pv_h[oh],
                        )
                    nc.sync.dma_start(v_dst(h, jv), vlocal[:, jj, :])
                nc.gpsimd.collective_compute(
                    "AllGather",
                    ALU.bypass,
                    replica_groups=RG,
                    ins=[kv_loc_h[h].opt()],
                    outs=[kv_full_h[h].opt()],
                )

        # -- local Q^T (overlaps the collectives) --
        wq_pool = p1.enter_context(tc.tile_pool(name="wq", bufs=1))
        wqt = wq_pool.tile([P, DO, D], F32R)
        _transpose_weight(nc, tp_psum, row_pool, id_r, w_q, wqt)
        for oo in range(DO):
            pq_h = [
                mm_psum.tile([P, JBLK], F32, tag="mm", name="pq") for _ in range(2)
            ]
            for dd in range(DO):
                for ih in range(2):
                    nc.tensor.matmul(
                        pq_h[ih],
                        (wqt[:, dd, oo * P : (oo + 1) * P]),
                        (xt[:, dd, ih * JBLK : (ih + 1) * JBLK]),
                        start=(dd == 0),
                        stop=(dd == DO - 1),
                    )
            for ih in range(2):
                nc.scalar.activation(
                    qt[:, oo, ih * JBLK : (ih + 1) * JBLK],
                    pq_h[ih],
                    AF.Identity,
                    bias=bq_sb[:, oo : oo + 1],
                )

    # =========================================================
    # Phase 2: streamed attention in E^T layout, ring order.
    # Block order: own shard (both halves, from kv_loc, no AG dep),
    # then ranks (pid+s) mod 8 for s=1..7, off=0 half then off=512.
    # =========================================================
    with ExitStack() as p2:
        oa_pool = p2.enter_context(tc.tile_pool(name="oacc", bufs=1))
        outacc = oa_pool.tile([P, IC, D], F32)  # 4 MB

        ktb_bf_pool = p2.enter_context(tc.tile_pool(name="ktbb", bufs=2))
        kt_pool = p2.enter_context(tc.tile_pool(name="ktb", bufs=2))
        v_pool = p2.enter_context(tc.tile_pool(name="vtb", bufs=3))
        pt_pool = p2.enter_context(tc.tile_pool(name="ptb", bufs=3))
        e_psum = p2.enter_context(tc.tile_pool(name="e_ps", bufs=4, space="PSUM"))
        o_psum = p2.enter_context(tc.tile_pool(name="o_ps", bufs=3, space="PSUM"))
        s_psum = p2.enter_context(tc.tile_pool(name="s_ps", bufs=1, space="PSUM"))

        fin_pool = p2.enter_context(tc.tile_pool(name="fin", bufs=2))
        pid = nc.sync.partition_id()

        blocks = [("loc", 0, 0), ("loc", 1, 0)]
        blocks += [("rem", h, s) for h in (0, 1) for s in range(1, NCORES)]

        for t, (kind, h, s) in enumerate(blocks):
            ktb = kt_pool.tile([P, DO, JBLK], F32R, tag="ktb")
            if kind == "loc":
                # own shard: upcast straight out of SBUF, no DMA at all
                for oo in range(DO):
                    nc.vector.tensor_copy(
                        out=ktb[:, oo, :],
                        in_=klocal[:, oo, h * JBLK : (h + 1) * JBLK],
                    )
                vtb = vlocal[:, h * (JBLK // P) : (h + 1) * (JBLK // P), :]
            else:
                rr = (pid + s) & (NCORES - 1)
                k_src = kv_full_h[h][ds(rr * (2 * HALF), HALF)].rearrange(
                    "(a p r) -> p a r", a=DO, p=P
                )
                v_src = kv_full_h[h][ds(rr * (2 * HALF) + HALF, HALF)].rearrange(
                    "(jj p o) -> p jj o", p=P, o=D
                )
                ktb_bf = ktb_bf_pool.tile([P, DO, JBLK], BF16, tag="ktbb")
                nc.sync.dma_start(ktb_bf, k_src)
                for oo in range(DO):
                    nc.vector.tensor_copy(out=ktb[:, oo, :], in_=ktb_bf[:, oo, :])
                vtb = v_pool.tile([P, JBLK // P, D], BF16, tag="vtb")
                nc.sync.dma_start(vtb, v_src)
            # unnormalized probabilities P^T for this j-block: [j, i]
            ptb = pt_pool.tile([P, JBLK // P, R], BF16, tag="ptb")
            for jj in range(JBLK // P):
                pe_h = [
                    e_psum.tile([P, JBLK], F32, tag="pe", name="pe")
                    for _ in range(R // JBLK)
                ]
                for oo in range(DO):
                    for ih in range(R // JBLK):
                        nc.tensor.matmul(
                            pe_h[ih],
                            (ktb[:, oo, jj * P : (jj + 1) * P]),
                            (qt[:, oo, ih * JBLK : (ih + 1) * JBLK]),
                            start=(oo == 0),
                            stop=(oo == DO - 1),
                        )
                for ih in range(R // JBLK):
                    nc.scalar.activation(
                        ptb[:, jj, ih * JBLK : (ih + 1) * JBLK],
                        pe_h[ih],
                        AF.Exp,
                        scale=SCALE,
                    )
            # row sums of P^T (reduce over j): matmul against ones
            # out_unnorm += P^T.T @ V, with the exp-sums matmul sharing each
            # stationary ptb tile (3 streams per weight load)
            last = t == len(blocks) - 1
            ps = s_psum.tile([P, 2 * IC], F32, tag="ps")
            for ic in range(IC):
                po_h = [o_psum.tile([P, 512], F32, tag="po", name="po") for _ in range(2)]
                for jj in range(JBLK // P):
                    for oh in range(2):
                        nc.tensor.matmul(
                            po_h[oh],
                            (ptb[:, jj, ic * P : (ic + 1) * P]),
                            (vtb[:, jj, oh * 512 : (oh + 1) * 512]),
                            start=(jj == 0),
                            stop=(jj == JBLK // P - 1),
                        )
                    # sums group: whole-block on most blocks; per-ic on the
                    # last block so the epilogue pipelines with the PE
                    nc.tensor.matmul(
                        ps[:, 2 * ic : 2 * ic + 2],
                        (ptb[:, jj, ic * P : (ic + 1) * P]),
                        (ones),
                        start=(jj == 0) if last else (ic == 0 and jj == 0),
                        stop=(jj == JBLK // P - 1)
                        if last
                        else (ic == IC - 1 and jj == JBLK // P - 1),
                    )
                for oh in range(2):
                    dst = outacc[:, ic, oh * 512 : (oh + 1) * 512]
                    if t == 0:
                        nc.vector.tensor_copy(out=dst, in_=po_h[oh])
                    else:
                        nc.vector.tensor_tensor(dst, po_h[oh], dst, ALU.add)
                if last:
                    # finalize rows [ic*128, (ic+1)*128) while the PE moves on
                    nc.vector.tensor_tensor(
                        sums_acc[:, 2 * ic : 2 * ic + 2],
                        ps[:, 2 * ic : 2 * ic + 2],
                        sums_acc[:, 2 * ic : 2 * ic + 2],
                        ALU.add,
                    )
                    nc.vector.reciprocal(
                        rsum[:, 2 * ic : 2 * ic + 2],
                        sums_acc[:, 2 * ic : 2 * ic + 2],
                    )
                    ofin = fin_pool.tile([P, D], F32, tag="ofin", name="ofin")
                    nc.vector.tensor_scalar_mul(
                        ofin, outacc[:, ic, :], rsum[:, 2 * ic : 2 * ic + 1]
                    )
                    nc.vector.tensor_tensor(ofin, ofin, bv_bc, ALU.add)
                    nc.sync.dma_start(out_loc[ic * P : (ic + 1) * P, :], ofin)
            if not last:
                if t == 0:
                    nc.vector.tensor_copy(out=sums_acc, in_=ps)
                else:
                    nc.vector.tensor_tensor(sums_acc, ps, sums_acc, ALU.add)

    outer.close()


_NC_CACHE = None


def _get_program():
    global _NC_CACHE
    if _NC_CACHE is None:
        _NC_CACHE = build_program()
    return _NC_CACHE


def _run(inputs, trace=False):
    nc = _get_program()
    x = np.ascontiguousarray(np.asarray(inputs["x"], dtype=np.float32))
    common = {
        k: np.ascontiguousarray(np.asarray(inputs[k], dtype=np.float32))
        for k in ("Wq", "Wk", "Wv", "bq", "bk", "bv")
    }
    in_maps = [
        {"x_loc": np.ascontiguousarray(x[c * R : (c + 1) * R]), **common}
        for c in range(NCORES)
    ]
    try:
        res = run_bass_kernel_spmd(
            nc, in_maps, core_ids=list(range(NCORES)), trace=trace
        )
    except Exception:
        # transient NRT_EXEC_UNIT_UNRECOVERABLE wedges recover on retry
        import time as _time

        _time.sleep(2)
        res = run_bass_kernel_spmd(
            nc, in_maps, core_ids=list(range(NCORES)), trace=trace
        )
    out = np.concatenate([res.results[c]["out_loc"] for c in range(NCORES)], axis=0)
    return out.reshape(B, D, 1).astype(np.float32), res


def kernel(**inputs):
    out, _ = _run(inputs, trace=False)
    return out
